# revision 12
# baseline (speedup 1.0000x reference)
"""ContinuousKoopman kernel for Trainium2 (8 NeuronCores).

Computes out = inputs @ expm(Dt * K) where K is a 64x64 tridiagonal matrix
built from diag/off_diags, and inputs is (524288, 64) float32.

Strategy:
  - expm(Dt*K) is a tiny 64x64 computation: done on host in float64
    (Pade-13 scaling-and-squaring, same family as the reference), cast to f32.
  - The heavy part (524288, 64) @ (64, 64) is data-parallel: the batch dim is
    sharded 8 ways; each NeuronCore processes 65536 rows (16 MiB in/out).
  - Per core, x rows are processed in pairs of 128-row chunks:
      * one PE transpose turns a [128, 128] tile (= two 128x64 chunks side by
        side) into [128p, 128] = [xT_a; xT_b] stacked on partitions 0-63/64-127
      * two row-tiled matmuls (K=64 each) against E (stationary, duplicated on
        both partition halves) produce natural-layout [128, 64] outputs
      * psum -> sbuf copies, then contiguous DMA stores.
"""

import sys

import numpy as np

if "/opt/trn_rl_repo" not in sys.path:
    sys.path.insert(0, "/opt/trn_rl_repo")

# ---------------------------------------------------------------- host expm

_PADE13_B = (
    64764752532480000.0, 32382376266240000.0, 7771770303897600.0,
    1187353796428800.0, 129060195264000.0, 10559470521600.0,
    670442572800.0, 33522128640.0, 1323241920.0, 40840800.0,
    960960.0, 16380.0, 182.0, 1.0,
)
_NUM_SQUARINGS = 8


def _expm_pade13(A: np.ndarray) -> np.ndarray:
    """Matrix exponential via Pade-13 with fixed scaling-and-squaring (f64)."""
    b = _PADE13_B
    n = A.shape[0]
    A = A.astype(np.float64) * (1.0 / (2.0 ** _NUM_SQUARINGS))
    I = np.eye(n, dtype=np.float64)
    A2 = A @ A
    A4 = A2 @ A2
    A6 = A4 @ A2
    U = A @ (A6 @ (b[13] * A6 + b[11] * A4 + b[9] * A2)
             + b[7] * A6 + b[5] * A4 + b[3] * A2 + b[1] * I)
    V = (A6 @ (b[12] * A6 + b[10] * A4 + b[8] * A2)
         + b[6] * A6 + b[4] * A4 + b[2] * A2 + b[0] * I)
    R = np.linalg.solve(V - U, V + U)
    for _ in range(_NUM_SQUARINGS):
        R = R @ R
    return R


def _build_E(diag: np.ndarray, off_diags: np.ndarray, Dt) -> np.ndarray:
    d = diag.astype(np.float64)
    o = off_diags.astype(np.float64)
    K = np.diag(-np.square(d)) + np.diag(o, k=1) + np.diag(-o, k=-1)
    E = _expm_pade13(float(np.asarray(Dt)) * K)
    return E.astype(np.float32)


# ---------------------------------------------------------------- bass kernel

N_CORES = 8
BATCH = 524288
UNITS = 64
ROWS = BATCH // N_CORES          # 65536 rows per core
PAIRS = ROWS // 256              # 256 pairs of 128-row chunks
PAIRS_PER_SLAB = 16              # 16 pairs -> 1 MiB x-slab
SLABS = PAIRS // PAIRS_PER_SLAB  # 16 slabs

_CACHE = {}


def _build_nc(rows=ROWS, pairs_per_slab=PAIRS_PER_SLAB):
    import concourse.mybir as mybir
    from concourse import bacc
    from concourse.tile import TileContext

    f32 = mybir.dt.float32
    nc = bacc.Bacc("TRN2")

    x = nc.dram_tensor("x", [rows, UNITS], f32, kind="ExternalInput")
    e2 = nc.dram_tensor("e2", [128, UNITS], f32, kind="ExternalInput")
    ident = nc.dram_tensor("ident", [128, 128], f32, kind="ExternalInput")
    y = nc.dram_tensor("y", [rows, UNITS], f32, kind="ExternalOutput")

    # row index = ((s*PP + n)*2 + c)*128 + p ; element u
    PP = pairs_per_slab
    n_slabs = rows // (PP * 256)
    x_t = x.ap().rearrange("(s n c p) u -> s p n c u", s=n_slabs, n=PP, c=2, p=128)
    y_t = y.ap().rearrange("(s n c p) u -> s p n c u", s=n_slabs, n=PP, c=2, p=128)

    with TileContext(nc) as tc:
        with (
            tc.tile_pool(name="const", bufs=1) as const_pool,
            tc.tile_pool(name="xs", bufs=2) as x_pool,
            tc.tile_pool(name="os", bufs=2) as o_pool,
            tc.tile_pool(name="wt", bufs=4) as w_pool,
            tc.tile_pool(name="ps", bufs=4, space="PSUM") as psum_pool,
        ):
            e2_sb = const_pool.tile([128, UNITS], f32)
            nc.sync.dma_start(out=e2_sb, in_=e2.ap())
            ident_sb = const_pool.tile([128, 128], f32)
            nc.sync.dma_start(out=ident_sb, in_=ident.ap())

            for s in range(n_slabs):
                x_slab = x_pool.tile([128, PP * 128], f32)
                nc.sync.dma_start(
                    out=x_slab.rearrange("p (n c u) -> p n c u", n=PP, c=2, u=UNITS),
                    in_=x_t[s])
                out_slab = o_pool.tile([128, PP * 128], f32)
                for n in range(PP):
                    xp = x_slab[:, n * 128:(n + 1) * 128]
                    psT = psum_pool.tile([128, 128], f32)
                    nc.tensor.transpose(psT, xp, ident_sb)
                    wt = w_pool.tile([128, 128], f32)
                    nc.vector.tensor_copy(wt, psT)
                    psO = psum_pool.tile([128, 128], f32)
                    nc.tensor.matmul(psO[:, 0:64], wt[0:64, :], e2_sb[0:64, :],
                                     start=True, stop=True)
                    nc.tensor.matmul(psO[:, 64:128], wt[64:128, :], e2_sb[64:128, :],
                                     start=True, stop=True)
                    nc.any.tensor_copy(out_slab[:, n * 128:(n + 1) * 128], psO)
                nc.sync.dma_start(
                    out=y_t[s],
                    in_=out_slab.rearrange("p (n c u) -> p n c u", n=PP, c=2, u=UNITS))

    return nc


def _build_nc_v2(rows=ROWS, pairs_per_slab=32, x_bufs=3, o_bufs=3):
    """v2: 2-rows-per-partition interleave (512B DMA segments), blockdiag E
    single matmul per pair, batched psum->sbuf copies, loads on sync HWDGE
    ring + stores on scalar HWDGE ring.

    Layout: a "pair tile" [128, 128] holds 256 consecutive rows: partition p
    carries rows base+2p (free 0:64) and base+2p+1 (free 64:128), i.e. 512
    contiguous bytes of DRAM per partition.  Its PE transpose stacks the two
    interleaved chunks' x^T on partitions 0-63 / 64-127, and one matmul
    against blockdiag(E, E) produces the outputs for both rows in the same
    natural [128, 128] layout, stored back with the mirror-image AP.
    """
    import concourse.mybir as mybir
    from concourse import bacc
    from concourse.tile import TileContext

    f32 = mybir.dt.float32
    nc = bacc.Bacc("TRN2")

    x = nc.dram_tensor("x", [rows, UNITS], f32, kind="ExternalInput")
    eb = nc.dram_tensor("eb", [128, 128], f32, kind="ExternalInput")
    ident = nc.dram_tensor("ident", [128, 128], f32, kind="ExternalInput")
    y = nc.dram_tensor("y", [rows, UNITS], f32, kind="ExternalOutput")

    PP = pairs_per_slab
    n_slabs = rows // (PP * 256)
    assert n_slabs * PP * 256 == rows
    assert PP % 4 == 0

    # row = ((s*PP + n)*128 + p)*2 + r
    x_t = x.ap().rearrange("(s n p r) u -> s p n (r u)", s=n_slabs, n=PP, p=128, r=2)
    y_t = y.ap().rearrange("(s n p r) u -> s p n (r u)", s=n_slabs, n=PP, p=128, r=2)

    with TileContext(nc) as tc:
        with (
            tc.tile_pool(name="const", bufs=1) as const_pool,
            tc.tile_pool(name="xs", bufs=x_bufs) as x_pool,
            tc.tile_pool(name="os", bufs=o_bufs) as o_pool,
            tc.tile_pool(name="wt", bufs=4) as w_pool,
            tc.tile_pool(name="pst", bufs=3, space="PSUM") as psT_pool,
            tc.tile_pool(name="pso", bufs=3, space="PSUM") as psO_pool,
        ):
            eb_sb = const_pool.tile([128, 128], f32)
            nc.sync.dma_start(out=eb_sb, in_=eb.ap())
            ident_sb = const_pool.tile([128, 128], f32)
            nc.sync.dma_start(out=ident_sb, in_=ident.ap())

            for s in range(n_slabs):
                x_slab = x_pool.tile([128, PP * 128], f32)
                nc.sync.dma_start(out=x_slab, in_=x_t[s])
                out_slab = o_pool.tile([128, PP * 128], f32)
                for m in range(PP // 4):          # 4 pairs per psO bank
                    psO4 = psO_pool.tile([128, 512], f32)
                    for h in range(2):            # 2 pairs per psT half-bank
                        psT2 = psT_pool.tile([128, 256], f32)
                        for q in range(2):
                            n = m * 4 + h * 2 + q
                            nc.tensor.transpose(
                                psT2[:, q * 128:(q + 1) * 128],
                                x_slab[:, n * 128:(n + 1) * 128],
                                ident_sb)
                        wt2 = w_pool.tile([128, 256], f32)
                        nc.vector.tensor_copy(wt2, psT2)
                        for q in range(2):
                            nloc = h * 2 + q
                            nc.tensor.matmul(
                                psO4[:, nloc * 128:(nloc + 1) * 128],
                                wt2[:, q * 128:(q + 1) * 128],
                                eb_sb,
                                start=True, stop=True)
                    nc.scalar.copy(
                        out_slab[:, m * 512:(m + 1) * 512], psO4)
                nc.scalar.dma_start(out=y_t[s], in_=out_slab)

    return nc


def _build_nc_v3(rows=ROWS, pairs_per_slab=32, x_bufs=3, o_bufs=3):
    """v3: E = I + D decomposition.  out = x + x_f16 @ D_f16.

    Since Dt*K has tiny norm, D = E - I has entries ~1e-3, so the correction
    term x@D only needs ~f16 precision for ~1e-5 relative output error, while
    x itself passes through exactly (f32 add on DVE).  This turns the PE work
    into a single fp16 matmul per 256 rows (1 cyc/row + FWL weight loads)
    instead of the fp32 multi-pass path, and the transposes move to the DMA
    xbar (2-byte dtype), eliminating all psum->sbuf transpose copies.

    Per 4-pair group (1024 rows):
      - DVE cast:   x16 = f16(x_slab slice)          [128, 512]
      - xbar DMA:   wt16[p, q, f] = x16[f, q*128+p]  (4 block transposes)
      - PE:         psO4[:, q*128:+128] = wt16[:,q,:].T @ blockdiag(D, D)
      - DVE add:    out_slab slice = x_slab slice + psO4   (psum read fused)
    """
    import concourse.mybir as mybir
    from concourse import bacc
    from concourse.tile import TileContext

    f32 = mybir.dt.float32
    f16 = mybir.dt.float16
    nc = bacc.Bacc("TRN2")

    x = nc.dram_tensor("x", [rows, UNITS], f32, kind="ExternalInput")
    db = nc.dram_tensor("db", [128, 128], f16, kind="ExternalInput")
    y = nc.dram_tensor("y", [rows, UNITS], f32, kind="ExternalOutput")

    PP = pairs_per_slab
    n_slabs = rows // (PP * 256)
    assert n_slabs * PP * 256 == rows
    assert PP % 4 == 0

    # row = ((s*PP + n)*128 + p)*2 + r
    x_t = x.ap().rearrange("(s n p r) u -> s p n (r u)", s=n_slabs, n=PP, p=128, r=2)
    y_t = y.ap().rearrange("(s n p r) u -> s p n (r u)", s=n_slabs, n=PP, p=128, r=2)

    with TileContext(nc) as tc:
        with (
            tc.tile_pool(name="const", bufs=1) as const_pool,
            tc.tile_pool(name="xs", bufs=x_bufs) as x_pool,
            tc.tile_pool(name="os", bufs=o_bufs) as o_pool,
            tc.tile_pool(name="x16", bufs=4) as x16_pool,
            tc.tile_pool(name="wt", bufs=4) as w_pool,
            tc.tile_pool(name="pso", bufs=4, space="PSUM") as psO_pool,
        ):
            db_sb = const_pool.tile([128, 128], f16)
            nc.sync.dma_start(out=db_sb, in_=db.ap())

            for s in range(n_slabs):
                x_slab = x_pool.tile([128, PP * 128], f32)
                nc.sync.dma_start(out=x_slab, in_=x_t[s])
                out_slab = o_pool.tile([128, PP * 128], f32)
                for m in range(PP // 4):          # 4 pairs per group
                    xsl = x_slab[:, m * 512:(m + 1) * 512]
                    x16 = x16_pool.tile([128, 512], f16)
                    nc.vector.tensor_copy(x16, xsl)
                    wt16 = w_pool.tile([128, 512], f16)
                    nc.sync.dma_start_transpose(
                        wt16.rearrange("p (q f) -> p q f", q=4), x16)
                    psO4 = psO_pool.tile([128, 512], f32)
                    for q in range(4):
                        nc.tensor.matmul(
                            psO4[:, q * 128:(q + 1) * 128],
                            wt16[:, q * 128:(q + 1) * 128],
                            db_sb,
                            start=True, stop=True)
                    nc.vector.tensor_add(
                        out_slab[:, m * 512:(m + 1) * 512], xsl, psO4)
                nc.scalar.dma_start(out=y_t[s], in_=out_slab)

    return nc


VARIANT = 3


def _get_nc():
    if "nc" not in _CACHE:
        nc = _build_nc_v3() if VARIANT == 3 else _build_nc_v2()
        nc.finalize()
        _CACHE["nc"] = nc
    return _CACHE["nc"]


def kernel(inputs, diag, off_diags, Dt, _trace=False):
    from concourse.bass_utils import run_bass_kernel_spmd

    E = _build_E(diag, off_diags, Dt)                       # (64, 64) f32
    x = np.ascontiguousarray(inputs.astype(np.float32, copy=False))
    nc = _get_nc()

    if VARIANT == 3:
        D = E.astype(np.float64) - np.eye(64)
        db_np = np.zeros((128, 128), dtype=np.float16)      # blockdiag(D, D)
        db_np[:64, :64] = D.astype(np.float16)
        db_np[64:, 64:] = D.astype(np.float16)
        in_maps = [
            {"x": x[i * ROWS:(i + 1) * ROWS], "db": db_np}
            for i in range(N_CORES)
        ]
    else:
        eb_np = np.zeros((128, 128), dtype=np.float32)      # blockdiag(E, E)
        eb_np[:64, :64] = E
        eb_np[64:, 64:] = E
        ident_np = np.eye(128, dtype=np.float32)
        in_maps = [
            {"x": x[i * ROWS:(i + 1) * ROWS], "eb": eb_np, "ident": ident_np}
            for i in range(N_CORES)
        ]

    res = run_bass_kernel_spmd(nc, in_maps, core_ids=list(range(N_CORES)),
                               trace=_trace)
    out = np.concatenate([r["y"] for r in res.results], axis=0)
    if _trace:
        _CACHE["last_results"] = res
    return out


# revision 15
# speedup vs baseline: 1.6136x; 1.6136x over previous
"""ContinuousKoopman kernel for Trainium2 (8 NeuronCores).

Computes out = inputs @ expm(Dt * K) where K is a 64x64 tridiagonal matrix
built from diag/off_diags, and inputs is (524288, 64) float32.

Strategy:
  - expm(Dt*K) is a tiny 64x64 computation: done on host in float64
    (Pade-13 scaling-and-squaring, same family as the reference), cast to f32.
  - The heavy part (524288, 64) @ (64, 64) is data-parallel: the batch dim is
    sharded 8 ways; each NeuronCore processes 65536 rows (16 MiB in/out).
  - Per core, x rows are processed in pairs of 128-row chunks:
      * one PE transpose turns a [128, 128] tile (= two 128x64 chunks side by
        side) into [128p, 128] = [xT_a; xT_b] stacked on partitions 0-63/64-127
      * two row-tiled matmuls (K=64 each) against E (stationary, duplicated on
        both partition halves) produce natural-layout [128, 64] outputs
      * psum -> sbuf copies, then contiguous DMA stores.
"""

import sys

import numpy as np

if "/opt/trn_rl_repo" not in sys.path:
    sys.path.insert(0, "/opt/trn_rl_repo")

# ---------------------------------------------------------------- host expm

_PADE13_B = (
    64764752532480000.0, 32382376266240000.0, 7771770303897600.0,
    1187353796428800.0, 129060195264000.0, 10559470521600.0,
    670442572800.0, 33522128640.0, 1323241920.0, 40840800.0,
    960960.0, 16380.0, 182.0, 1.0,
)
_NUM_SQUARINGS = 8


def _expm_pade13(A: np.ndarray) -> np.ndarray:
    """Matrix exponential via Pade-13 with fixed scaling-and-squaring (f64)."""
    b = _PADE13_B
    n = A.shape[0]
    A = A.astype(np.float64) * (1.0 / (2.0 ** _NUM_SQUARINGS))
    I = np.eye(n, dtype=np.float64)
    A2 = A @ A
    A4 = A2 @ A2
    A6 = A4 @ A2
    U = A @ (A6 @ (b[13] * A6 + b[11] * A4 + b[9] * A2)
             + b[7] * A6 + b[5] * A4 + b[3] * A2 + b[1] * I)
    V = (A6 @ (b[12] * A6 + b[10] * A4 + b[8] * A2)
         + b[6] * A6 + b[4] * A4 + b[2] * A2 + b[0] * I)
    R = np.linalg.solve(V - U, V + U)
    for _ in range(_NUM_SQUARINGS):
        R = R @ R
    return R


def _build_E(diag: np.ndarray, off_diags: np.ndarray, Dt) -> np.ndarray:
    d = diag.astype(np.float64)
    o = off_diags.astype(np.float64)
    K = np.diag(-np.square(d)) + np.diag(o, k=1) + np.diag(-o, k=-1)
    E = _expm_pade13(float(np.asarray(Dt)) * K)
    return E.astype(np.float32)


# ---------------------------------------------------------------- bass kernel

N_CORES = 8
BATCH = 524288
UNITS = 64
ROWS = BATCH // N_CORES          # 65536 rows per core
PAIRS = ROWS // 256              # 256 pairs of 128-row chunks
PAIRS_PER_SLAB = 16              # 16 pairs -> 1 MiB x-slab
SLABS = PAIRS // PAIRS_PER_SLAB  # 16 slabs

_CACHE = {}


def _build_nc(rows=ROWS, pairs_per_slab=PAIRS_PER_SLAB):
    import concourse.mybir as mybir
    from concourse import bacc
    from concourse.tile import TileContext

    f32 = mybir.dt.float32
    nc = bacc.Bacc("TRN2")

    x = nc.dram_tensor("x", [rows, UNITS], f32, kind="ExternalInput")
    e2 = nc.dram_tensor("e2", [128, UNITS], f32, kind="ExternalInput")
    ident = nc.dram_tensor("ident", [128, 128], f32, kind="ExternalInput")
    y = nc.dram_tensor("y", [rows, UNITS], f32, kind="ExternalOutput")

    # row index = ((s*PP + n)*2 + c)*128 + p ; element u
    PP = pairs_per_slab
    n_slabs = rows // (PP * 256)
    x_t = x.ap().rearrange("(s n c p) u -> s p n c u", s=n_slabs, n=PP, c=2, p=128)
    y_t = y.ap().rearrange("(s n c p) u -> s p n c u", s=n_slabs, n=PP, c=2, p=128)

    with TileContext(nc) as tc:
        with (
            tc.tile_pool(name="const", bufs=1) as const_pool,
            tc.tile_pool(name="xs", bufs=2) as x_pool,
            tc.tile_pool(name="os", bufs=2) as o_pool,
            tc.tile_pool(name="wt", bufs=4) as w_pool,
            tc.tile_pool(name="ps", bufs=4, space="PSUM") as psum_pool,
        ):
            e2_sb = const_pool.tile([128, UNITS], f32)
            nc.sync.dma_start(out=e2_sb, in_=e2.ap())
            ident_sb = const_pool.tile([128, 128], f32)
            nc.sync.dma_start(out=ident_sb, in_=ident.ap())

            for s in range(n_slabs):
                x_slab = x_pool.tile([128, PP * 128], f32)
                nc.sync.dma_start(
                    out=x_slab.rearrange("p (n c u) -> p n c u", n=PP, c=2, u=UNITS),
                    in_=x_t[s])
                out_slab = o_pool.tile([128, PP * 128], f32)
                for n in range(PP):
                    xp = x_slab[:, n * 128:(n + 1) * 128]
                    psT = psum_pool.tile([128, 128], f32)
                    nc.tensor.transpose(psT, xp, ident_sb)
                    wt = w_pool.tile([128, 128], f32)
                    nc.vector.tensor_copy(wt, psT)
                    psO = psum_pool.tile([128, 128], f32)
                    nc.tensor.matmul(psO[:, 0:64], wt[0:64, :], e2_sb[0:64, :],
                                     start=True, stop=True)
                    nc.tensor.matmul(psO[:, 64:128], wt[64:128, :], e2_sb[64:128, :],
                                     start=True, stop=True)
                    nc.any.tensor_copy(out_slab[:, n * 128:(n + 1) * 128], psO)
                nc.sync.dma_start(
                    out=y_t[s],
                    in_=out_slab.rearrange("p (n c u) -> p n c u", n=PP, c=2, u=UNITS))

    return nc


def _build_nc_v2(rows=ROWS, pairs_per_slab=32, x_bufs=3, o_bufs=3):
    """v2: 2-rows-per-partition interleave (512B DMA segments), blockdiag E
    single matmul per pair, batched psum->sbuf copies, loads on sync HWDGE
    ring + stores on scalar HWDGE ring.

    Layout: a "pair tile" [128, 128] holds 256 consecutive rows: partition p
    carries rows base+2p (free 0:64) and base+2p+1 (free 64:128), i.e. 512
    contiguous bytes of DRAM per partition.  Its PE transpose stacks the two
    interleaved chunks' x^T on partitions 0-63 / 64-127, and one matmul
    against blockdiag(E, E) produces the outputs for both rows in the same
    natural [128, 128] layout, stored back with the mirror-image AP.
    """
    import concourse.mybir as mybir
    from concourse import bacc
    from concourse.tile import TileContext

    f32 = mybir.dt.float32
    nc = bacc.Bacc("TRN2")

    x = nc.dram_tensor("x", [rows, UNITS], f32, kind="ExternalInput")
    eb = nc.dram_tensor("eb", [128, 128], f32, kind="ExternalInput")
    ident = nc.dram_tensor("ident", [128, 128], f32, kind="ExternalInput")
    y = nc.dram_tensor("y", [rows, UNITS], f32, kind="ExternalOutput")

    PP = pairs_per_slab
    n_slabs = rows // (PP * 256)
    assert n_slabs * PP * 256 == rows
    assert PP % 4 == 0

    # row = ((s*PP + n)*128 + p)*2 + r
    x_t = x.ap().rearrange("(s n p r) u -> s p n (r u)", s=n_slabs, n=PP, p=128, r=2)
    y_t = y.ap().rearrange("(s n p r) u -> s p n (r u)", s=n_slabs, n=PP, p=128, r=2)

    with TileContext(nc) as tc:
        with (
            tc.tile_pool(name="const", bufs=1) as const_pool,
            tc.tile_pool(name="xs", bufs=x_bufs) as x_pool,
            tc.tile_pool(name="os", bufs=o_bufs) as o_pool,
            tc.tile_pool(name="wt", bufs=4) as w_pool,
            tc.tile_pool(name="pst", bufs=3, space="PSUM") as psT_pool,
            tc.tile_pool(name="pso", bufs=3, space="PSUM") as psO_pool,
        ):
            eb_sb = const_pool.tile([128, 128], f32)
            nc.sync.dma_start(out=eb_sb, in_=eb.ap())
            ident_sb = const_pool.tile([128, 128], f32)
            nc.sync.dma_start(out=ident_sb, in_=ident.ap())

            for s in range(n_slabs):
                x_slab = x_pool.tile([128, PP * 128], f32)
                nc.sync.dma_start(out=x_slab, in_=x_t[s])
                out_slab = o_pool.tile([128, PP * 128], f32)
                for m in range(PP // 4):          # 4 pairs per psO bank
                    psO4 = psO_pool.tile([128, 512], f32)
                    for h in range(2):            # 2 pairs per psT half-bank
                        psT2 = psT_pool.tile([128, 256], f32)
                        for q in range(2):
                            n = m * 4 + h * 2 + q
                            nc.tensor.transpose(
                                psT2[:, q * 128:(q + 1) * 128],
                                x_slab[:, n * 128:(n + 1) * 128],
                                ident_sb)
                        wt2 = w_pool.tile([128, 256], f32)
                        nc.vector.tensor_copy(wt2, psT2)
                        for q in range(2):
                            nloc = h * 2 + q
                            nc.tensor.matmul(
                                psO4[:, nloc * 128:(nloc + 1) * 128],
                                wt2[:, q * 128:(q + 1) * 128],
                                eb_sb,
                                start=True, stop=True)
                    nc.scalar.copy(
                        out_slab[:, m * 512:(m + 1) * 512], psO4)
                nc.scalar.dma_start(out=y_t[s], in_=out_slab)

    return nc


def _build_nc_v3(rows=ROWS, pairs_per_slab=32, x_bufs=3, o_bufs=3):
    """v3: E = I + D decomposition.  out = x + x_f16 @ D_f16.

    Since Dt*K has tiny norm, D = E - I has entries ~1e-3, so the correction
    term x@D only needs ~f16 precision for ~1e-5 relative output error, while
    x itself passes through exactly (f32 add on DVE).  This turns the PE work
    into a single fp16 matmul per 256 rows (1 cyc/row + FWL weight loads)
    instead of the fp32 multi-pass path, and the transposes move to the DMA
    xbar (2-byte dtype), eliminating all psum->sbuf transpose copies.

    Per 4-pair group (1024 rows):
      - DVE cast:   x16 = f16(x_slab slice)          [128, 512]
      - xbar DMA:   wt16[p, q, f] = x16[f, q*128+p]  (4 block transposes)
      - PE:         psO4[:, q*128:+128] = wt16[:,q,:].T @ blockdiag(D, D)
      - DVE add:    out_slab slice = x_slab slice + psO4   (psum read fused)
    """
    import concourse.mybir as mybir
    from concourse import bacc
    from concourse.tile import TileContext

    f32 = mybir.dt.float32
    f16 = mybir.dt.float16
    nc = bacc.Bacc("TRN2")

    x = nc.dram_tensor("x", [rows, UNITS], f32, kind="ExternalInput")
    db = nc.dram_tensor("db", [128, 128], f16, kind="ExternalInput")
    y = nc.dram_tensor("y", [rows, UNITS], f32, kind="ExternalOutput")

    PP = pairs_per_slab
    n_slabs = rows // (PP * 256)
    assert n_slabs * PP * 256 == rows
    assert PP % 4 == 0

    # row = ((s*PP + n)*128 + p)*2 + r
    x_t = x.ap().rearrange("(s n p r) u -> s p n (r u)", s=n_slabs, n=PP, p=128, r=2)
    y_t = y.ap().rearrange("(s n p r) u -> s p n (r u)", s=n_slabs, n=PP, p=128, r=2)

    with TileContext(nc) as tc:
        with (
            tc.tile_pool(name="const", bufs=1) as const_pool,
            tc.tile_pool(name="xs", bufs=x_bufs) as x_pool,
            tc.tile_pool(name="os", bufs=o_bufs) as o_pool,
            tc.tile_pool(name="x16", bufs=4) as x16_pool,
            tc.tile_pool(name="wt", bufs=4) as w_pool,
            tc.tile_pool(name="pso", bufs=4, space="PSUM") as psO_pool,
        ):
            db_sb = const_pool.tile([128, 128], f16)
            nc.sync.dma_start(out=db_sb, in_=db.ap())

            for s in range(n_slabs):
                x_slab = x_pool.tile([128, PP * 128], f32)
                nc.sync.dma_start(out=x_slab, in_=x_t[s])
                out_slab = o_pool.tile([128, PP * 128], f32)
                for m in range(PP // 4):          # 4 pairs per group
                    xsl = x_slab[:, m * 512:(m + 1) * 512]
                    x16 = x16_pool.tile([128, 512], f16)
                    nc.vector.tensor_copy(x16, xsl)
                    wt16 = w_pool.tile([128, 512], f16)
                    nc.sync.dma_start_transpose(
                        wt16.rearrange("p (q f) -> p q f", q=4), x16)
                    psO4 = psO_pool.tile([128, 512], f32)
                    for q in range(4):
                        nc.tensor.matmul(
                            psO4[:, q * 128:(q + 1) * 128],
                            wt16[:, q * 128:(q + 1) * 128],
                            db_sb,
                            start=True, stop=True)
                    nc.vector.tensor_add(
                        out_slab[:, m * 512:(m + 1) * 512], xsl, psO4)
                nc.scalar.dma_start(out=y_t[s], in_=out_slab)

    return nc


def _build_nc_v3b(rows=ROWS, pairs_per_slab=32, x_bufs=3, o_bufs=3):
    """v3b: like v3 (out = x + x_f16 @ D_f16) but transposes on the PE in
    f16 (1 cyc/row) instead of the DMA xbar (which serialized the sync
    queue and added 16 MiB of fabric traffic).  Engine assignment spreads
    the elementwise work: GpSimd casts, ScalarE copies the transposed f16
    out of psum, VectorE does the fused x + psum add.
    """
    import concourse.mybir as mybir
    from concourse import bacc
    from concourse.tile import TileContext

    f32 = mybir.dt.float32
    f16 = mybir.dt.float16
    nc = bacc.Bacc("TRN2")

    x = nc.dram_tensor("x", [rows, UNITS], f32, kind="ExternalInput")
    db = nc.dram_tensor("db", [128, 128], f16, kind="ExternalInput")
    ident = nc.dram_tensor("ident", [128, 128], f16, kind="ExternalInput")
    y = nc.dram_tensor("y", [rows, UNITS], f32, kind="ExternalOutput")

    PP = pairs_per_slab
    n_slabs = rows // (PP * 256)
    assert n_slabs * PP * 256 == rows
    assert PP % 4 == 0

    # row = ((s*PP + n)*128 + p)*2 + r
    x_t = x.ap().rearrange("(s n p r) u -> s p n (r u)", s=n_slabs, n=PP, p=128, r=2)
    y_t = y.ap().rearrange("(s n p r) u -> s p n (r u)", s=n_slabs, n=PP, p=128, r=2)

    with TileContext(nc) as tc:
        with (
            tc.tile_pool(name="const", bufs=1) as const_pool,
            tc.tile_pool(name="xs", bufs=x_bufs) as x_pool,
            tc.tile_pool(name="os", bufs=o_bufs) as o_pool,
            tc.tile_pool(name="x16", bufs=4) as x16_pool,
            tc.tile_pool(name="wt", bufs=4) as w_pool,
            tc.tile_pool(name="pst", bufs=3, space="PSUM") as psT_pool,
            tc.tile_pool(name="pso", bufs=3, space="PSUM") as psO_pool,
        ):
            db_sb = const_pool.tile([128, 128], f16)
            nc.sync.dma_start(out=db_sb, in_=db.ap())
            ident_sb = const_pool.tile([128, 128], f16)
            nc.sync.dma_start(out=ident_sb, in_=ident.ap())

            for s in range(n_slabs):
                x_slab = x_pool.tile([128, PP * 128], f32)
                nc.sync.dma_start(out=x_slab, in_=x_t[s])
                out_slab = o_pool.tile([128, PP * 128], f32)
                for m in range(PP // 4):          # 4 pairs per group
                    xsl = x_slab[:, m * 512:(m + 1) * 512]
                    x16 = x16_pool.tile([128, 512], f16)
                    nc.gpsimd.tensor_copy(x16, xsl)
                    psT16 = psT_pool.tile([128, 512], f16)
                    for q in range(4):
                        nc.tensor.transpose(
                            psT16[:, q * 128:(q + 1) * 128],
                            x16[:, q * 128:(q + 1) * 128],
                            ident_sb)
                    wt16 = w_pool.tile([128, 512], f16)
                    nc.scalar.copy(wt16, psT16)
                    psO4 = psO_pool.tile([128, 512], f32)
                    for q in range(4):
                        nc.tensor.matmul(
                            psO4[:, q * 128:(q + 1) * 128],
                            wt16[:, q * 128:(q + 1) * 128],
                            db_sb,
                            start=True, stop=True)
                    nc.vector.tensor_add(
                        out_slab[:, m * 512:(m + 1) * 512], xsl, psO4)
                nc.scalar.dma_start(out=y_t[s], in_=out_slab)

    return nc


VARIANT = "3b"


_BUILDERS = {2: _build_nc_v2, 3: _build_nc_v3}


def _get_nc():
    if "nc" not in _CACHE:
        nc = _BUILDERS.get(VARIANT, _build_nc_v3b)()
        nc.finalize()
        _CACHE["nc"] = nc
    return _CACHE["nc"]


def kernel(inputs, diag, off_diags, Dt, _trace=False):
    from concourse.bass_utils import run_bass_kernel_spmd

    E = _build_E(diag, off_diags, Dt)                       # (64, 64) f32
    x = np.ascontiguousarray(inputs.astype(np.float32, copy=False))
    nc = _get_nc()

    if VARIANT in (3, "3b"):
        D = E.astype(np.float64) - np.eye(64)
        db_np = np.zeros((128, 128), dtype=np.float16)      # blockdiag(D, D)
        db_np[:64, :64] = D.astype(np.float16)
        db_np[64:, 64:] = D.astype(np.float16)
        in_maps = [
            {"x": x[i * ROWS:(i + 1) * ROWS], "db": db_np}
            for i in range(N_CORES)
        ]
        if VARIANT == "3b":
            ident_np = np.eye(128, dtype=np.float16)
            for m in in_maps:
                m["ident"] = ident_np
    else:
        eb_np = np.zeros((128, 128), dtype=np.float32)      # blockdiag(E, E)
        eb_np[:64, :64] = E
        eb_np[64:, 64:] = E
        ident_np = np.eye(128, dtype=np.float32)
        in_maps = [
            {"x": x[i * ROWS:(i + 1) * ROWS], "eb": eb_np, "ident": ident_np}
            for i in range(N_CORES)
        ]

    res = run_bass_kernel_spmd(nc, in_maps, core_ids=list(range(N_CORES)),
                               trace=_trace)
    out = np.concatenate([r["y"] for r in res.results], axis=0)
    if _trace:
        _CACHE["last_results"] = res
    return out


# revision 16
# speedup vs baseline: 2.0281x; 1.2569x over previous
"""ContinuousKoopman kernel for Trainium2 (8 NeuronCores).

Computes out = inputs @ expm(Dt * K) where K is a 64x64 tridiagonal matrix
built from diag/off_diags, and inputs is (524288, 64) float32.

Strategy:
  - expm(Dt*K) is a tiny 64x64 computation: done on host in float64
    (Pade-13 scaling-and-squaring, same family as the reference), cast to f32.
  - The heavy part (524288, 64) @ (64, 64) is data-parallel: the batch dim is
    sharded 8 ways; each NeuronCore processes 65536 rows (16 MiB in/out).
  - Per core, x rows are processed in pairs of 128-row chunks:
      * one PE transpose turns a [128, 128] tile (= two 128x64 chunks side by
        side) into [128p, 128] = [xT_a; xT_b] stacked on partitions 0-63/64-127
      * two row-tiled matmuls (K=64 each) against E (stationary, duplicated on
        both partition halves) produce natural-layout [128, 64] outputs
      * psum -> sbuf copies, then contiguous DMA stores.
"""

import sys

import numpy as np

if "/opt/trn_rl_repo" not in sys.path:
    sys.path.insert(0, "/opt/trn_rl_repo")

# ---------------------------------------------------------------- host expm

_PADE13_B = (
    64764752532480000.0, 32382376266240000.0, 7771770303897600.0,
    1187353796428800.0, 129060195264000.0, 10559470521600.0,
    670442572800.0, 33522128640.0, 1323241920.0, 40840800.0,
    960960.0, 16380.0, 182.0, 1.0,
)
_NUM_SQUARINGS = 8


def _expm_pade13(A: np.ndarray) -> np.ndarray:
    """Matrix exponential via Pade-13 with fixed scaling-and-squaring (f64)."""
    b = _PADE13_B
    n = A.shape[0]
    A = A.astype(np.float64) * (1.0 / (2.0 ** _NUM_SQUARINGS))
    I = np.eye(n, dtype=np.float64)
    A2 = A @ A
    A4 = A2 @ A2
    A6 = A4 @ A2
    U = A @ (A6 @ (b[13] * A6 + b[11] * A4 + b[9] * A2)
             + b[7] * A6 + b[5] * A4 + b[3] * A2 + b[1] * I)
    V = (A6 @ (b[12] * A6 + b[10] * A4 + b[8] * A2)
         + b[6] * A6 + b[4] * A4 + b[2] * A2 + b[0] * I)
    R = np.linalg.solve(V - U, V + U)
    for _ in range(_NUM_SQUARINGS):
        R = R @ R
    return R


def _build_E(diag: np.ndarray, off_diags: np.ndarray, Dt) -> np.ndarray:
    d = diag.astype(np.float64)
    o = off_diags.astype(np.float64)
    K = np.diag(-np.square(d)) + np.diag(o, k=1) + np.diag(-o, k=-1)
    E = _expm_pade13(float(np.asarray(Dt)) * K)
    return E.astype(np.float32)


# ---------------------------------------------------------------- bass kernel

N_CORES = 8
BATCH = 524288
UNITS = 64
ROWS = BATCH // N_CORES          # 65536 rows per core
PAIRS = ROWS // 256              # 256 pairs of 128-row chunks
PAIRS_PER_SLAB = 16              # 16 pairs -> 1 MiB x-slab
SLABS = PAIRS // PAIRS_PER_SLAB  # 16 slabs

_CACHE = {}


def _build_nc(rows=ROWS, pairs_per_slab=PAIRS_PER_SLAB):
    import concourse.mybir as mybir
    from concourse import bacc
    from concourse.tile import TileContext

    f32 = mybir.dt.float32
    nc = bacc.Bacc("TRN2")

    x = nc.dram_tensor("x", [rows, UNITS], f32, kind="ExternalInput")
    e2 = nc.dram_tensor("e2", [128, UNITS], f32, kind="ExternalInput")
    ident = nc.dram_tensor("ident", [128, 128], f32, kind="ExternalInput")
    y = nc.dram_tensor("y", [rows, UNITS], f32, kind="ExternalOutput")

    # row index = ((s*PP + n)*2 + c)*128 + p ; element u
    PP = pairs_per_slab
    n_slabs = rows // (PP * 256)
    x_t = x.ap().rearrange("(s n c p) u -> s p n c u", s=n_slabs, n=PP, c=2, p=128)
    y_t = y.ap().rearrange("(s n c p) u -> s p n c u", s=n_slabs, n=PP, c=2, p=128)

    with TileContext(nc) as tc:
        with (
            tc.tile_pool(name="const", bufs=1) as const_pool,
            tc.tile_pool(name="xs", bufs=2) as x_pool,
            tc.tile_pool(name="os", bufs=2) as o_pool,
            tc.tile_pool(name="wt", bufs=4) as w_pool,
            tc.tile_pool(name="ps", bufs=4, space="PSUM") as psum_pool,
        ):
            e2_sb = const_pool.tile([128, UNITS], f32)
            nc.sync.dma_start(out=e2_sb, in_=e2.ap())
            ident_sb = const_pool.tile([128, 128], f32)
            nc.sync.dma_start(out=ident_sb, in_=ident.ap())

            for s in range(n_slabs):
                x_slab = x_pool.tile([128, PP * 128], f32)
                nc.sync.dma_start(
                    out=x_slab.rearrange("p (n c u) -> p n c u", n=PP, c=2, u=UNITS),
                    in_=x_t[s])
                out_slab = o_pool.tile([128, PP * 128], f32)
                for n in range(PP):
                    xp = x_slab[:, n * 128:(n + 1) * 128]
                    psT = psum_pool.tile([128, 128], f32)
                    nc.tensor.transpose(psT, xp, ident_sb)
                    wt = w_pool.tile([128, 128], f32)
                    nc.vector.tensor_copy(wt, psT)
                    psO = psum_pool.tile([128, 128], f32)
                    nc.tensor.matmul(psO[:, 0:64], wt[0:64, :], e2_sb[0:64, :],
                                     start=True, stop=True)
                    nc.tensor.matmul(psO[:, 64:128], wt[64:128, :], e2_sb[64:128, :],
                                     start=True, stop=True)
                    nc.any.tensor_copy(out_slab[:, n * 128:(n + 1) * 128], psO)
                nc.sync.dma_start(
                    out=y_t[s],
                    in_=out_slab.rearrange("p (n c u) -> p n c u", n=PP, c=2, u=UNITS))

    return nc


def _build_nc_v2(rows=ROWS, pairs_per_slab=32, x_bufs=3, o_bufs=3):
    """v2: 2-rows-per-partition interleave (512B DMA segments), blockdiag E
    single matmul per pair, batched psum->sbuf copies, loads on sync HWDGE
    ring + stores on scalar HWDGE ring.

    Layout: a "pair tile" [128, 128] holds 256 consecutive rows: partition p
    carries rows base+2p (free 0:64) and base+2p+1 (free 64:128), i.e. 512
    contiguous bytes of DRAM per partition.  Its PE transpose stacks the two
    interleaved chunks' x^T on partitions 0-63 / 64-127, and one matmul
    against blockdiag(E, E) produces the outputs for both rows in the same
    natural [128, 128] layout, stored back with the mirror-image AP.
    """
    import concourse.mybir as mybir
    from concourse import bacc
    from concourse.tile import TileContext

    f32 = mybir.dt.float32
    nc = bacc.Bacc("TRN2")

    x = nc.dram_tensor("x", [rows, UNITS], f32, kind="ExternalInput")
    eb = nc.dram_tensor("eb", [128, 128], f32, kind="ExternalInput")
    ident = nc.dram_tensor("ident", [128, 128], f32, kind="ExternalInput")
    y = nc.dram_tensor("y", [rows, UNITS], f32, kind="ExternalOutput")

    PP = pairs_per_slab
    n_slabs = rows // (PP * 256)
    assert n_slabs * PP * 256 == rows
    assert PP % 4 == 0

    # row = ((s*PP + n)*128 + p)*2 + r
    x_t = x.ap().rearrange("(s n p r) u -> s p n (r u)", s=n_slabs, n=PP, p=128, r=2)
    y_t = y.ap().rearrange("(s n p r) u -> s p n (r u)", s=n_slabs, n=PP, p=128, r=2)

    with TileContext(nc) as tc:
        with (
            tc.tile_pool(name="const", bufs=1) as const_pool,
            tc.tile_pool(name="xs", bufs=x_bufs) as x_pool,
            tc.tile_pool(name="os", bufs=o_bufs) as o_pool,
            tc.tile_pool(name="wt", bufs=4) as w_pool,
            tc.tile_pool(name="pst", bufs=3, space="PSUM") as psT_pool,
            tc.tile_pool(name="pso", bufs=3, space="PSUM") as psO_pool,
        ):
            eb_sb = const_pool.tile([128, 128], f32)
            nc.sync.dma_start(out=eb_sb, in_=eb.ap())
            ident_sb = const_pool.tile([128, 128], f32)
            nc.sync.dma_start(out=ident_sb, in_=ident.ap())

            for s in range(n_slabs):
                x_slab = x_pool.tile([128, PP * 128], f32)
                nc.sync.dma_start(out=x_slab, in_=x_t[s])
                out_slab = o_pool.tile([128, PP * 128], f32)
                for m in range(PP // 4):          # 4 pairs per psO bank
                    psO4 = psO_pool.tile([128, 512], f32)
                    for h in range(2):            # 2 pairs per psT half-bank
                        psT2 = psT_pool.tile([128, 256], f32)
                        for q in range(2):
                            n = m * 4 + h * 2 + q
                            nc.tensor.transpose(
                                psT2[:, q * 128:(q + 1) * 128],
                                x_slab[:, n * 128:(n + 1) * 128],
                                ident_sb)
                        wt2 = w_pool.tile([128, 256], f32)
                        nc.vector.tensor_copy(wt2, psT2)
                        for q in range(2):
                            nloc = h * 2 + q
                            nc.tensor.matmul(
                                psO4[:, nloc * 128:(nloc + 1) * 128],
                                wt2[:, q * 128:(q + 1) * 128],
                                eb_sb,
                                start=True, stop=True)
                    nc.scalar.copy(
                        out_slab[:, m * 512:(m + 1) * 512], psO4)
                nc.scalar.dma_start(out=y_t[s], in_=out_slab)

    return nc


def _build_nc_v3(rows=ROWS, pairs_per_slab=32, x_bufs=3, o_bufs=3):
    """v3: E = I + D decomposition.  out = x + x_f16 @ D_f16.

    Since Dt*K has tiny norm, D = E - I has entries ~1e-3, so the correction
    term x@D only needs ~f16 precision for ~1e-5 relative output error, while
    x itself passes through exactly (f32 add on DVE).  This turns the PE work
    into a single fp16 matmul per 256 rows (1 cyc/row + FWL weight loads)
    instead of the fp32 multi-pass path, and the transposes move to the DMA
    xbar (2-byte dtype), eliminating all psum->sbuf transpose copies.

    Per 4-pair group (1024 rows):
      - DVE cast:   x16 = f16(x_slab slice)          [128, 512]
      - xbar DMA:   wt16[p, q, f] = x16[f, q*128+p]  (4 block transposes)
      - PE:         psO4[:, q*128:+128] = wt16[:,q,:].T @ blockdiag(D, D)
      - DVE add:    out_slab slice = x_slab slice + psO4   (psum read fused)
    """
    import concourse.mybir as mybir
    from concourse import bacc
    from concourse.tile import TileContext

    f32 = mybir.dt.float32
    f16 = mybir.dt.float16
    nc = bacc.Bacc("TRN2")

    x = nc.dram_tensor("x", [rows, UNITS], f32, kind="ExternalInput")
    db = nc.dram_tensor("db", [128, 128], f16, kind="ExternalInput")
    y = nc.dram_tensor("y", [rows, UNITS], f32, kind="ExternalOutput")

    PP = pairs_per_slab
    n_slabs = rows // (PP * 256)
    assert n_slabs * PP * 256 == rows
    assert PP % 4 == 0

    # row = ((s*PP + n)*128 + p)*2 + r
    x_t = x.ap().rearrange("(s n p r) u -> s p n (r u)", s=n_slabs, n=PP, p=128, r=2)
    y_t = y.ap().rearrange("(s n p r) u -> s p n (r u)", s=n_slabs, n=PP, p=128, r=2)

    with TileContext(nc) as tc:
        with (
            tc.tile_pool(name="const", bufs=1) as const_pool,
            tc.tile_pool(name="xs", bufs=x_bufs) as x_pool,
            tc.tile_pool(name="os", bufs=o_bufs) as o_pool,
            tc.tile_pool(name="x16", bufs=4) as x16_pool,
            tc.tile_pool(name="wt", bufs=4) as w_pool,
            tc.tile_pool(name="pso", bufs=4, space="PSUM") as psO_pool,
        ):
            db_sb = const_pool.tile([128, 128], f16)
            nc.sync.dma_start(out=db_sb, in_=db.ap())

            for s in range(n_slabs):
                x_slab = x_pool.tile([128, PP * 128], f32)
                nc.sync.dma_start(out=x_slab, in_=x_t[s])
                out_slab = o_pool.tile([128, PP * 128], f32)
                for m in range(PP // 4):          # 4 pairs per group
                    xsl = x_slab[:, m * 512:(m + 1) * 512]
                    x16 = x16_pool.tile([128, 512], f16)
                    nc.vector.tensor_copy(x16, xsl)
                    wt16 = w_pool.tile([128, 512], f16)
                    nc.sync.dma_start_transpose(
                        wt16.rearrange("p (q f) -> p q f", q=4), x16)
                    psO4 = psO_pool.tile([128, 512], f32)
                    for q in range(4):
                        nc.tensor.matmul(
                            psO4[:, q * 128:(q + 1) * 128],
                            wt16[:, q * 128:(q + 1) * 128],
                            db_sb,
                            start=True, stop=True)
                    nc.vector.tensor_add(
                        out_slab[:, m * 512:(m + 1) * 512], xsl, psO4)
                nc.scalar.dma_start(out=y_t[s], in_=out_slab)

    return nc


def _build_nc_v3b(rows=ROWS, pairs_per_slab=32, x_bufs=3, o_bufs=3):
    """v3b: like v3 (out = x + x_f16 @ D_f16) but transposes on the PE in
    f16 (1 cyc/row) instead of the DMA xbar (which serialized the sync
    queue and added 16 MiB of fabric traffic).  Engine assignment spreads
    the elementwise work: GpSimd casts, ScalarE copies the transposed f16
    out of psum, VectorE does the fused x + psum add.
    """
    import concourse.mybir as mybir
    from concourse import bacc
    from concourse.tile import TileContext

    f32 = mybir.dt.float32
    f16 = mybir.dt.float16
    nc = bacc.Bacc("TRN2")

    x = nc.dram_tensor("x", [rows, UNITS], f32, kind="ExternalInput")
    db = nc.dram_tensor("db", [128, 128], f16, kind="ExternalInput")
    ident = nc.dram_tensor("ident", [128, 128], f16, kind="ExternalInput")
    y = nc.dram_tensor("y", [rows, UNITS], f32, kind="ExternalOutput")

    PP = pairs_per_slab
    n_slabs = rows // (PP * 256)
    assert n_slabs * PP * 256 == rows
    assert PP % 4 == 0

    # row = ((s*PP + n)*128 + p)*2 + r
    x_t = x.ap().rearrange("(s n p r) u -> s p n (r u)", s=n_slabs, n=PP, p=128, r=2)
    y_t = y.ap().rearrange("(s n p r) u -> s p n (r u)", s=n_slabs, n=PP, p=128, r=2)

    with TileContext(nc) as tc:
        with (
            tc.tile_pool(name="const", bufs=1) as const_pool,
            tc.tile_pool(name="xs", bufs=x_bufs) as x_pool,
            tc.tile_pool(name="os", bufs=o_bufs) as o_pool,
            tc.tile_pool(name="x16", bufs=4) as x16_pool,
            tc.tile_pool(name="wt", bufs=4) as w_pool,
            tc.tile_pool(name="pst", bufs=3, space="PSUM") as psT_pool,
            tc.tile_pool(name="pso", bufs=3, space="PSUM") as psO_pool,
        ):
            db_sb = const_pool.tile([128, 128], f16)
            nc.sync.dma_start(out=db_sb, in_=db.ap())
            ident_sb = const_pool.tile([128, 128], f16)
            nc.sync.dma_start(out=ident_sb, in_=ident.ap())

            for s in range(n_slabs):
                x_slab = x_pool.tile([128, PP * 128], f32)
                nc.sync.dma_start(out=x_slab, in_=x_t[s])
                out_slab = o_pool.tile([128, PP * 128], f32)
                for m in range(PP // 4):          # 4 pairs per group
                    xsl = x_slab[:, m * 512:(m + 1) * 512]
                    x16 = x16_pool.tile([128, 512], f16)
                    # GpSimd casts are ~3.5x slower than DVE; give it 1 in 4
                    cast_eng = nc.gpsimd if m % 4 == 3 else nc.vector
                    cast_eng.tensor_copy(x16, xsl)
                    psT16 = psT_pool.tile([128, 512], f16)
                    for q in range(4):
                        nc.tensor.transpose(
                            psT16[:, q * 128:(q + 1) * 128],
                            x16[:, q * 128:(q + 1) * 128],
                            ident_sb)
                    wt16 = w_pool.tile([128, 512], f16)
                    nc.scalar.copy(wt16, psT16)
                    psO4 = psO_pool.tile([128, 512], f32)
                    for q in range(4):
                        nc.tensor.matmul(
                            psO4[:, q * 128:(q + 1) * 128],
                            wt16[:, q * 128:(q + 1) * 128],
                            db_sb,
                            start=True, stop=True)
                    nc.vector.tensor_add(
                        out_slab[:, m * 512:(m + 1) * 512], xsl, psO4)
                nc.scalar.dma_start(out=y_t[s], in_=out_slab)

    return nc


VARIANT = "3b"


_BUILDERS = {2: _build_nc_v2, 3: _build_nc_v3}


def _get_nc():
    if "nc" not in _CACHE:
        nc = _BUILDERS.get(VARIANT, _build_nc_v3b)()
        nc.finalize()
        _CACHE["nc"] = nc
    return _CACHE["nc"]


def kernel(inputs, diag, off_diags, Dt, _trace=False):
    from concourse.bass_utils import run_bass_kernel_spmd

    E = _build_E(diag, off_diags, Dt)                       # (64, 64) f32
    x = np.ascontiguousarray(inputs.astype(np.float32, copy=False))
    nc = _get_nc()

    if VARIANT in (3, "3b"):
        D = E.astype(np.float64) - np.eye(64)
        db_np = np.zeros((128, 128), dtype=np.float16)      # blockdiag(D, D)
        db_np[:64, :64] = D.astype(np.float16)
        db_np[64:, 64:] = D.astype(np.float16)
        in_maps = [
            {"x": x[i * ROWS:(i + 1) * ROWS], "db": db_np}
            for i in range(N_CORES)
        ]
        if VARIANT == "3b":
            ident_np = np.eye(128, dtype=np.float16)
            for m in in_maps:
                m["ident"] = ident_np
    else:
        eb_np = np.zeros((128, 128), dtype=np.float32)      # blockdiag(E, E)
        eb_np[:64, :64] = E
        eb_np[64:, 64:] = E
        ident_np = np.eye(128, dtype=np.float32)
        in_maps = [
            {"x": x[i * ROWS:(i + 1) * ROWS], "eb": eb_np, "ident": ident_np}
            for i in range(N_CORES)
        ]

    res = run_bass_kernel_spmd(nc, in_maps, core_ids=list(range(N_CORES)),
                               trace=_trace)
    out = np.concatenate([r["y"] for r in res.results], axis=0)
    if _trace:
        _CACHE["last_results"] = res
    return out


# revision 17
# speedup vs baseline: 2.0720x; 1.0217x over previous
"""ContinuousKoopman kernel for Trainium2 (8 NeuronCores).

Computes out = inputs @ expm(Dt * K) where K is a 64x64 tridiagonal matrix
built from diag/off_diags, and inputs is (524288, 64) float32.

Strategy:
  - expm(Dt*K) is a tiny 64x64 computation: done on host in float64
    (Pade-13 scaling-and-squaring, same family as the reference), cast to f32.
  - The heavy part (524288, 64) @ (64, 64) is data-parallel: the batch dim is
    sharded 8 ways; each NeuronCore processes 65536 rows (16 MiB in/out).
  - Per core, x rows are processed in pairs of 128-row chunks:
      * one PE transpose turns a [128, 128] tile (= two 128x64 chunks side by
        side) into [128p, 128] = [xT_a; xT_b] stacked on partitions 0-63/64-127
      * two row-tiled matmuls (K=64 each) against E (stationary, duplicated on
        both partition halves) produce natural-layout [128, 64] outputs
      * psum -> sbuf copies, then contiguous DMA stores.
"""

import sys

import numpy as np

if "/opt/trn_rl_repo" not in sys.path:
    sys.path.insert(0, "/opt/trn_rl_repo")

# ---------------------------------------------------------------- host expm

_PADE13_B = (
    64764752532480000.0, 32382376266240000.0, 7771770303897600.0,
    1187353796428800.0, 129060195264000.0, 10559470521600.0,
    670442572800.0, 33522128640.0, 1323241920.0, 40840800.0,
    960960.0, 16380.0, 182.0, 1.0,
)
_NUM_SQUARINGS = 8


def _expm_pade13(A: np.ndarray) -> np.ndarray:
    """Matrix exponential via Pade-13 with fixed scaling-and-squaring (f64)."""
    b = _PADE13_B
    n = A.shape[0]
    A = A.astype(np.float64) * (1.0 / (2.0 ** _NUM_SQUARINGS))
    I = np.eye(n, dtype=np.float64)
    A2 = A @ A
    A4 = A2 @ A2
    A6 = A4 @ A2
    U = A @ (A6 @ (b[13] * A6 + b[11] * A4 + b[9] * A2)
             + b[7] * A6 + b[5] * A4 + b[3] * A2 + b[1] * I)
    V = (A6 @ (b[12] * A6 + b[10] * A4 + b[8] * A2)
         + b[6] * A6 + b[4] * A4 + b[2] * A2 + b[0] * I)
    R = np.linalg.solve(V - U, V + U)
    for _ in range(_NUM_SQUARINGS):
        R = R @ R
    return R


def _build_E(diag: np.ndarray, off_diags: np.ndarray, Dt) -> np.ndarray:
    d = diag.astype(np.float64)
    o = off_diags.astype(np.float64)
    K = np.diag(-np.square(d)) + np.diag(o, k=1) + np.diag(-o, k=-1)
    E = _expm_pade13(float(np.asarray(Dt)) * K)
    return E.astype(np.float32)


# ---------------------------------------------------------------- bass kernel

N_CORES = 8
BATCH = 524288
UNITS = 64
ROWS = BATCH // N_CORES          # 65536 rows per core
PAIRS = ROWS // 256              # 256 pairs of 128-row chunks
PAIRS_PER_SLAB = 16              # 16 pairs -> 1 MiB x-slab
SLABS = PAIRS // PAIRS_PER_SLAB  # 16 slabs

_CACHE = {}


def _build_nc(rows=ROWS, pairs_per_slab=PAIRS_PER_SLAB):
    import concourse.mybir as mybir
    from concourse import bacc
    from concourse.tile import TileContext

    f32 = mybir.dt.float32
    nc = bacc.Bacc("TRN2")

    x = nc.dram_tensor("x", [rows, UNITS], f32, kind="ExternalInput")
    e2 = nc.dram_tensor("e2", [128, UNITS], f32, kind="ExternalInput")
    ident = nc.dram_tensor("ident", [128, 128], f32, kind="ExternalInput")
    y = nc.dram_tensor("y", [rows, UNITS], f32, kind="ExternalOutput")

    # row index = ((s*PP + n)*2 + c)*128 + p ; element u
    PP = pairs_per_slab
    n_slabs = rows // (PP * 256)
    x_t = x.ap().rearrange("(s n c p) u -> s p n c u", s=n_slabs, n=PP, c=2, p=128)
    y_t = y.ap().rearrange("(s n c p) u -> s p n c u", s=n_slabs, n=PP, c=2, p=128)

    with TileContext(nc) as tc:
        with (
            tc.tile_pool(name="const", bufs=1) as const_pool,
            tc.tile_pool(name="xs", bufs=2) as x_pool,
            tc.tile_pool(name="os", bufs=2) as o_pool,
            tc.tile_pool(name="wt", bufs=4) as w_pool,
            tc.tile_pool(name="ps", bufs=4, space="PSUM") as psum_pool,
        ):
            e2_sb = const_pool.tile([128, UNITS], f32)
            nc.sync.dma_start(out=e2_sb, in_=e2.ap())
            ident_sb = const_pool.tile([128, 128], f32)
            nc.sync.dma_start(out=ident_sb, in_=ident.ap())

            for s in range(n_slabs):
                x_slab = x_pool.tile([128, PP * 128], f32)
                nc.sync.dma_start(
                    out=x_slab.rearrange("p (n c u) -> p n c u", n=PP, c=2, u=UNITS),
                    in_=x_t[s])
                out_slab = o_pool.tile([128, PP * 128], f32)
                for n in range(PP):
                    xp = x_slab[:, n * 128:(n + 1) * 128]
                    psT = psum_pool.tile([128, 128], f32)
                    nc.tensor.transpose(psT, xp, ident_sb)
                    wt = w_pool.tile([128, 128], f32)
                    nc.vector.tensor_copy(wt, psT)
                    psO = psum_pool.tile([128, 128], f32)
                    nc.tensor.matmul(psO[:, 0:64], wt[0:64, :], e2_sb[0:64, :],
                                     start=True, stop=True)
                    nc.tensor.matmul(psO[:, 64:128], wt[64:128, :], e2_sb[64:128, :],
                                     start=True, stop=True)
                    nc.any.tensor_copy(out_slab[:, n * 128:(n + 1) * 128], psO)
                nc.sync.dma_start(
                    out=y_t[s],
                    in_=out_slab.rearrange("p (n c u) -> p n c u", n=PP, c=2, u=UNITS))

    return nc


def _build_nc_v2(rows=ROWS, pairs_per_slab=32, x_bufs=3, o_bufs=3):
    """v2: 2-rows-per-partition interleave (512B DMA segments), blockdiag E
    single matmul per pair, batched psum->sbuf copies, loads on sync HWDGE
    ring + stores on scalar HWDGE ring.

    Layout: a "pair tile" [128, 128] holds 256 consecutive rows: partition p
    carries rows base+2p (free 0:64) and base+2p+1 (free 64:128), i.e. 512
    contiguous bytes of DRAM per partition.  Its PE transpose stacks the two
    interleaved chunks' x^T on partitions 0-63 / 64-127, and one matmul
    against blockdiag(E, E) produces the outputs for both rows in the same
    natural [128, 128] layout, stored back with the mirror-image AP.
    """
    import concourse.mybir as mybir
    from concourse import bacc
    from concourse.tile import TileContext

    f32 = mybir.dt.float32
    nc = bacc.Bacc("TRN2")

    x = nc.dram_tensor("x", [rows, UNITS], f32, kind="ExternalInput")
    eb = nc.dram_tensor("eb", [128, 128], f32, kind="ExternalInput")
    ident = nc.dram_tensor("ident", [128, 128], f32, kind="ExternalInput")
    y = nc.dram_tensor("y", [rows, UNITS], f32, kind="ExternalOutput")

    PP = pairs_per_slab
    n_slabs = rows // (PP * 256)
    assert n_slabs * PP * 256 == rows
    assert PP % 4 == 0

    # row = ((s*PP + n)*128 + p)*2 + r
    x_t = x.ap().rearrange("(s n p r) u -> s p n (r u)", s=n_slabs, n=PP, p=128, r=2)
    y_t = y.ap().rearrange("(s n p r) u -> s p n (r u)", s=n_slabs, n=PP, p=128, r=2)

    with TileContext(nc) as tc:
        with (
            tc.tile_pool(name="const", bufs=1) as const_pool,
            tc.tile_pool(name="xs", bufs=x_bufs) as x_pool,
            tc.tile_pool(name="os", bufs=o_bufs) as o_pool,
            tc.tile_pool(name="wt", bufs=4) as w_pool,
            tc.tile_pool(name="pst", bufs=3, space="PSUM") as psT_pool,
            tc.tile_pool(name="pso", bufs=3, space="PSUM") as psO_pool,
        ):
            eb_sb = const_pool.tile([128, 128], f32)
            nc.sync.dma_start(out=eb_sb, in_=eb.ap())
            ident_sb = const_pool.tile([128, 128], f32)
            nc.sync.dma_start(out=ident_sb, in_=ident.ap())

            for s in range(n_slabs):
                x_slab = x_pool.tile([128, PP * 128], f32)
                nc.sync.dma_start(out=x_slab, in_=x_t[s])
                out_slab = o_pool.tile([128, PP * 128], f32)
                for m in range(PP // 4):          # 4 pairs per psO bank
                    psO4 = psO_pool.tile([128, 512], f32)
                    for h in range(2):            # 2 pairs per psT half-bank
                        psT2 = psT_pool.tile([128, 256], f32)
                        for q in range(2):
                            n = m * 4 + h * 2 + q
                            nc.tensor.transpose(
                                psT2[:, q * 128:(q + 1) * 128],
                                x_slab[:, n * 128:(n + 1) * 128],
                                ident_sb)
                        wt2 = w_pool.tile([128, 256], f32)
                        nc.vector.tensor_copy(wt2, psT2)
                        for q in range(2):
                            nloc = h * 2 + q
                            nc.tensor.matmul(
                                psO4[:, nloc * 128:(nloc + 1) * 128],
                                wt2[:, q * 128:(q + 1) * 128],
                                eb_sb,
                                start=True, stop=True)
                    nc.scalar.copy(
                        out_slab[:, m * 512:(m + 1) * 512], psO4)
                nc.scalar.dma_start(out=y_t[s], in_=out_slab)

    return nc


def _build_nc_v3(rows=ROWS, pairs_per_slab=32, x_bufs=3, o_bufs=3):
    """v3: E = I + D decomposition.  out = x + x_f16 @ D_f16.

    Since Dt*K has tiny norm, D = E - I has entries ~1e-3, so the correction
    term x@D only needs ~f16 precision for ~1e-5 relative output error, while
    x itself passes through exactly (f32 add on DVE).  This turns the PE work
    into a single fp16 matmul per 256 rows (1 cyc/row + FWL weight loads)
    instead of the fp32 multi-pass path, and the transposes move to the DMA
    xbar (2-byte dtype), eliminating all psum->sbuf transpose copies.

    Per 4-pair group (1024 rows):
      - DVE cast:   x16 = f16(x_slab slice)          [128, 512]
      - xbar DMA:   wt16[p, q, f] = x16[f, q*128+p]  (4 block transposes)
      - PE:         psO4[:, q*128:+128] = wt16[:,q,:].T @ blockdiag(D, D)
      - DVE add:    out_slab slice = x_slab slice + psO4   (psum read fused)
    """
    import concourse.mybir as mybir
    from concourse import bacc
    from concourse.tile import TileContext

    f32 = mybir.dt.float32
    f16 = mybir.dt.float16
    nc = bacc.Bacc("TRN2")

    x = nc.dram_tensor("x", [rows, UNITS], f32, kind="ExternalInput")
    db = nc.dram_tensor("db", [128, 128], f16, kind="ExternalInput")
    y = nc.dram_tensor("y", [rows, UNITS], f32, kind="ExternalOutput")

    PP = pairs_per_slab
    n_slabs = rows // (PP * 256)
    assert n_slabs * PP * 256 == rows
    assert PP % 4 == 0

    # row = ((s*PP + n)*128 + p)*2 + r
    x_t = x.ap().rearrange("(s n p r) u -> s p n (r u)", s=n_slabs, n=PP, p=128, r=2)
    y_t = y.ap().rearrange("(s n p r) u -> s p n (r u)", s=n_slabs, n=PP, p=128, r=2)

    with TileContext(nc) as tc:
        with (
            tc.tile_pool(name="const", bufs=1) as const_pool,
            tc.tile_pool(name="xs", bufs=x_bufs) as x_pool,
            tc.tile_pool(name="os", bufs=o_bufs) as o_pool,
            tc.tile_pool(name="x16", bufs=4) as x16_pool,
            tc.tile_pool(name="wt", bufs=4) as w_pool,
            tc.tile_pool(name="pso", bufs=4, space="PSUM") as psO_pool,
        ):
            db_sb = const_pool.tile([128, 128], f16)
            nc.sync.dma_start(out=db_sb, in_=db.ap())

            for s in range(n_slabs):
                x_slab = x_pool.tile([128, PP * 128], f32)
                nc.sync.dma_start(out=x_slab, in_=x_t[s])
                out_slab = o_pool.tile([128, PP * 128], f32)
                for m in range(PP // 4):          # 4 pairs per group
                    xsl = x_slab[:, m * 512:(m + 1) * 512]
                    x16 = x16_pool.tile([128, 512], f16)
                    nc.vector.tensor_copy(x16, xsl)
                    wt16 = w_pool.tile([128, 512], f16)
                    nc.sync.dma_start_transpose(
                        wt16.rearrange("p (q f) -> p q f", q=4), x16)
                    psO4 = psO_pool.tile([128, 512], f32)
                    for q in range(4):
                        nc.tensor.matmul(
                            psO4[:, q * 128:(q + 1) * 128],
                            wt16[:, q * 128:(q + 1) * 128],
                            db_sb,
                            start=True, stop=True)
                    nc.vector.tensor_add(
                        out_slab[:, m * 512:(m + 1) * 512], xsl, psO4)
                nc.scalar.dma_start(out=y_t[s], in_=out_slab)

    return nc


def _build_nc_v3b(rows=ROWS, pairs_per_slab=16, x_bufs=6, o_bufs=4):
    """v3b: like v3 (out = x + x_f16 @ D_f16) but transposes on the PE in
    f16 (1 cyc/row) instead of the DMA xbar (which serialized the sync
    queue and added 16 MiB of fabric traffic).  Engine assignment spreads
    the elementwise work: GpSimd casts, ScalarE copies the transposed f16
    out of psum, VectorE does the fused x + psum add.
    """
    import concourse.mybir as mybir
    from concourse import bacc
    from concourse.tile import TileContext

    f32 = mybir.dt.float32
    f16 = mybir.dt.float16
    nc = bacc.Bacc("TRN2")

    x = nc.dram_tensor("x", [rows, UNITS], f32, kind="ExternalInput")
    db = nc.dram_tensor("db", [128, 128], f16, kind="ExternalInput")
    ident = nc.dram_tensor("ident", [128, 128], f16, kind="ExternalInput")
    y = nc.dram_tensor("y", [rows, UNITS], f32, kind="ExternalOutput")

    PP = pairs_per_slab
    n_slabs = rows // (PP * 256)
    assert n_slabs * PP * 256 == rows
    assert PP % 4 == 0

    # row = ((s*PP + n)*128 + p)*2 + r
    x_t = x.ap().rearrange("(s n p r) u -> s p n (r u)", s=n_slabs, n=PP, p=128, r=2)
    y_t = y.ap().rearrange("(s n p r) u -> s p n (r u)", s=n_slabs, n=PP, p=128, r=2)

    with TileContext(nc) as tc:
        with (
            tc.tile_pool(name="const", bufs=1) as const_pool,
            tc.tile_pool(name="xs", bufs=x_bufs) as x_pool,
            tc.tile_pool(name="os", bufs=o_bufs) as o_pool,
            tc.tile_pool(name="x16", bufs=4) as x16_pool,
            tc.tile_pool(name="wt", bufs=4) as w_pool,
            tc.tile_pool(name="pst", bufs=3, space="PSUM") as psT_pool,
            tc.tile_pool(name="pso", bufs=3, space="PSUM") as psO_pool,
        ):
            db_sb = const_pool.tile([128, 128], f16)
            nc.sync.dma_start(out=db_sb, in_=db.ap())
            ident_sb = const_pool.tile([128, 128], f16)
            nc.sync.dma_start(out=ident_sb, in_=ident.ap())

            for s in range(n_slabs):
                x_slab = x_pool.tile([128, PP * 128], f32)
                nc.sync.dma_start(out=x_slab, in_=x_t[s])
                out_slab = o_pool.tile([128, PP * 128], f32)
                for m in range(PP // 4):          # 4 pairs per group
                    xsl = x_slab[:, m * 512:(m + 1) * 512]
                    x16 = x16_pool.tile([128, 512], f16)
                    # GpSimd casts are ~3.5x slower than DVE; give it 1 in 4
                    cast_eng = nc.gpsimd if m % 4 == 3 else nc.vector
                    cast_eng.tensor_copy(x16, xsl)
                    psT16 = psT_pool.tile([128, 512], f16)
                    for q in range(4):
                        nc.tensor.transpose(
                            psT16[:, q * 128:(q + 1) * 128],
                            x16[:, q * 128:(q + 1) * 128],
                            ident_sb)
                    wt16 = w_pool.tile([128, 512], f16)
                    nc.scalar.copy(wt16, psT16)
                    psO4 = psO_pool.tile([128, 512], f32)
                    for q in range(4):
                        nc.tensor.matmul(
                            psO4[:, q * 128:(q + 1) * 128],
                            wt16[:, q * 128:(q + 1) * 128],
                            db_sb,
                            start=True, stop=True)
                    nc.vector.tensor_add(
                        out_slab[:, m * 512:(m + 1) * 512], xsl, psO4)
                nc.scalar.dma_start(out=y_t[s], in_=out_slab)

    return nc


VARIANT = "3b"


_BUILDERS = {2: _build_nc_v2, 3: _build_nc_v3}


def _get_nc():
    if "nc" not in _CACHE:
        nc = _BUILDERS.get(VARIANT, _build_nc_v3b)()
        nc.finalize()
        _CACHE["nc"] = nc
    return _CACHE["nc"]


def kernel(inputs, diag, off_diags, Dt, _trace=False):
    from concourse.bass_utils import run_bass_kernel_spmd

    E = _build_E(diag, off_diags, Dt)                       # (64, 64) f32
    x = np.ascontiguousarray(inputs.astype(np.float32, copy=False))
    nc = _get_nc()

    if VARIANT in (3, "3b"):
        D = E.astype(np.float64) - np.eye(64)
        db_np = np.zeros((128, 128), dtype=np.float16)      # blockdiag(D, D)
        db_np[:64, :64] = D.astype(np.float16)
        db_np[64:, 64:] = D.astype(np.float16)
        in_maps = [
            {"x": x[i * ROWS:(i + 1) * ROWS], "db": db_np}
            for i in range(N_CORES)
        ]
        if VARIANT == "3b":
            ident_np = np.eye(128, dtype=np.float16)
            for m in in_maps:
                m["ident"] = ident_np
    else:
        eb_np = np.zeros((128, 128), dtype=np.float32)      # blockdiag(E, E)
        eb_np[:64, :64] = E
        eb_np[64:, 64:] = E
        ident_np = np.eye(128, dtype=np.float32)
        in_maps = [
            {"x": x[i * ROWS:(i + 1) * ROWS], "eb": eb_np, "ident": ident_np}
            for i in range(N_CORES)
        ]

    res = run_bass_kernel_spmd(nc, in_maps, core_ids=list(range(N_CORES)),
                               trace=_trace)
    out = np.concatenate([r["y"] for r in res.results], axis=0)
    if _trace:
        _CACHE["last_results"] = res
    return out


# revision 18
# speedup vs baseline: 2.3872x; 1.1521x over previous
"""ContinuousKoopman kernel for Trainium2 (8 NeuronCores).

Computes out = inputs @ expm(Dt * K) where K is a 64x64 tridiagonal matrix
built from diag/off_diags, and inputs is (524288, 64) float32.

Strategy:
  - expm(Dt*K) is a tiny 64x64 computation: done on host in float64
    (Pade-13 scaling-and-squaring, same family as the reference), cast to f32.
  - The heavy part (524288, 64) @ (64, 64) is data-parallel: the batch dim is
    sharded 8 ways; each NeuronCore processes 65536 rows (16 MiB in/out).
  - Per core, x rows are processed in pairs of 128-row chunks:
      * one PE transpose turns a [128, 128] tile (= two 128x64 chunks side by
        side) into [128p, 128] = [xT_a; xT_b] stacked on partitions 0-63/64-127
      * two row-tiled matmuls (K=64 each) against E (stationary, duplicated on
        both partition halves) produce natural-layout [128, 64] outputs
      * psum -> sbuf copies, then contiguous DMA stores.
"""

import sys

import numpy as np

if "/opt/trn_rl_repo" not in sys.path:
    sys.path.insert(0, "/opt/trn_rl_repo")

# ---------------------------------------------------------------- host expm

_PADE13_B = (
    64764752532480000.0, 32382376266240000.0, 7771770303897600.0,
    1187353796428800.0, 129060195264000.0, 10559470521600.0,
    670442572800.0, 33522128640.0, 1323241920.0, 40840800.0,
    960960.0, 16380.0, 182.0, 1.0,
)
_NUM_SQUARINGS = 8


def _expm_pade13(A: np.ndarray) -> np.ndarray:
    """Matrix exponential via Pade-13 with fixed scaling-and-squaring (f64)."""
    b = _PADE13_B
    n = A.shape[0]
    A = A.astype(np.float64) * (1.0 / (2.0 ** _NUM_SQUARINGS))
    I = np.eye(n, dtype=np.float64)
    A2 = A @ A
    A4 = A2 @ A2
    A6 = A4 @ A2
    U = A @ (A6 @ (b[13] * A6 + b[11] * A4 + b[9] * A2)
             + b[7] * A6 + b[5] * A4 + b[3] * A2 + b[1] * I)
    V = (A6 @ (b[12] * A6 + b[10] * A4 + b[8] * A2)
         + b[6] * A6 + b[4] * A4 + b[2] * A2 + b[0] * I)
    R = np.linalg.solve(V - U, V + U)
    for _ in range(_NUM_SQUARINGS):
        R = R @ R
    return R


def _build_E(diag: np.ndarray, off_diags: np.ndarray, Dt) -> np.ndarray:
    d = diag.astype(np.float64)
    o = off_diags.astype(np.float64)
    K = np.diag(-np.square(d)) + np.diag(o, k=1) + np.diag(-o, k=-1)
    E = _expm_pade13(float(np.asarray(Dt)) * K)
    return E.astype(np.float32)


# ---------------------------------------------------------------- bass kernel

N_CORES = 8
BATCH = 524288
UNITS = 64
ROWS = BATCH // N_CORES          # 65536 rows per core
PAIRS = ROWS // 256              # 256 pairs of 128-row chunks
PAIRS_PER_SLAB = 16              # 16 pairs -> 1 MiB x-slab
SLABS = PAIRS // PAIRS_PER_SLAB  # 16 slabs

_CACHE = {}


def _build_nc(rows=ROWS, pairs_per_slab=PAIRS_PER_SLAB):
    import concourse.mybir as mybir
    from concourse import bacc
    from concourse.tile import TileContext

    f32 = mybir.dt.float32
    nc = bacc.Bacc("TRN2")

    x = nc.dram_tensor("x", [rows, UNITS], f32, kind="ExternalInput")
    e2 = nc.dram_tensor("e2", [128, UNITS], f32, kind="ExternalInput")
    ident = nc.dram_tensor("ident", [128, 128], f32, kind="ExternalInput")
    y = nc.dram_tensor("y", [rows, UNITS], f32, kind="ExternalOutput")

    # row index = ((s*PP + n)*2 + c)*128 + p ; element u
    PP = pairs_per_slab
    n_slabs = rows // (PP * 256)
    x_t = x.ap().rearrange("(s n c p) u -> s p n c u", s=n_slabs, n=PP, c=2, p=128)
    y_t = y.ap().rearrange("(s n c p) u -> s p n c u", s=n_slabs, n=PP, c=2, p=128)

    with TileContext(nc) as tc:
        with (
            tc.tile_pool(name="const", bufs=1) as const_pool,
            tc.tile_pool(name="xs", bufs=2) as x_pool,
            tc.tile_pool(name="os", bufs=2) as o_pool,
            tc.tile_pool(name="wt", bufs=4) as w_pool,
            tc.tile_pool(name="ps", bufs=4, space="PSUM") as psum_pool,
        ):
            e2_sb = const_pool.tile([128, UNITS], f32)
            nc.sync.dma_start(out=e2_sb, in_=e2.ap())
            ident_sb = const_pool.tile([128, 128], f32)
            nc.sync.dma_start(out=ident_sb, in_=ident.ap())

            for s in range(n_slabs):
                x_slab = x_pool.tile([128, PP * 128], f32)
                nc.sync.dma_start(
                    out=x_slab.rearrange("p (n c u) -> p n c u", n=PP, c=2, u=UNITS),
                    in_=x_t[s])
                out_slab = o_pool.tile([128, PP * 128], f32)
                for n in range(PP):
                    xp = x_slab[:, n * 128:(n + 1) * 128]
                    psT = psum_pool.tile([128, 128], f32)
                    nc.tensor.transpose(psT, xp, ident_sb)
                    wt = w_pool.tile([128, 128], f32)
                    nc.vector.tensor_copy(wt, psT)
                    psO = psum_pool.tile([128, 128], f32)
                    nc.tensor.matmul(psO[:, 0:64], wt[0:64, :], e2_sb[0:64, :],
                                     start=True, stop=True)
                    nc.tensor.matmul(psO[:, 64:128], wt[64:128, :], e2_sb[64:128, :],
                                     start=True, stop=True)
                    nc.any.tensor_copy(out_slab[:, n * 128:(n + 1) * 128], psO)
                nc.sync.dma_start(
                    out=y_t[s],
                    in_=out_slab.rearrange("p (n c u) -> p n c u", n=PP, c=2, u=UNITS))

    return nc


def _build_nc_v2(rows=ROWS, pairs_per_slab=32, x_bufs=3, o_bufs=3):
    """v2: 2-rows-per-partition interleave (512B DMA segments), blockdiag E
    single matmul per pair, batched psum->sbuf copies, loads on sync HWDGE
    ring + stores on scalar HWDGE ring.

    Layout: a "pair tile" [128, 128] holds 256 consecutive rows: partition p
    carries rows base+2p (free 0:64) and base+2p+1 (free 64:128), i.e. 512
    contiguous bytes of DRAM per partition.  Its PE transpose stacks the two
    interleaved chunks' x^T on partitions 0-63 / 64-127, and one matmul
    against blockdiag(E, E) produces the outputs for both rows in the same
    natural [128, 128] layout, stored back with the mirror-image AP.
    """
    import concourse.mybir as mybir
    from concourse import bacc
    from concourse.tile import TileContext

    f32 = mybir.dt.float32
    nc = bacc.Bacc("TRN2")

    x = nc.dram_tensor("x", [rows, UNITS], f32, kind="ExternalInput")
    eb = nc.dram_tensor("eb", [128, 128], f32, kind="ExternalInput")
    ident = nc.dram_tensor("ident", [128, 128], f32, kind="ExternalInput")
    y = nc.dram_tensor("y", [rows, UNITS], f32, kind="ExternalOutput")

    PP = pairs_per_slab
    n_slabs = rows // (PP * 256)
    assert n_slabs * PP * 256 == rows
    assert PP % 4 == 0

    # row = ((s*PP + n)*128 + p)*2 + r
    x_t = x.ap().rearrange("(s n p r) u -> s p n (r u)", s=n_slabs, n=PP, p=128, r=2)
    y_t = y.ap().rearrange("(s n p r) u -> s p n (r u)", s=n_slabs, n=PP, p=128, r=2)

    with TileContext(nc) as tc:
        with (
            tc.tile_pool(name="const", bufs=1) as const_pool,
            tc.tile_pool(name="xs", bufs=x_bufs) as x_pool,
            tc.tile_pool(name="os", bufs=o_bufs) as o_pool,
            tc.tile_pool(name="wt", bufs=4) as w_pool,
            tc.tile_pool(name="pst", bufs=3, space="PSUM") as psT_pool,
            tc.tile_pool(name="pso", bufs=3, space="PSUM") as psO_pool,
        ):
            eb_sb = const_pool.tile([128, 128], f32)
            nc.sync.dma_start(out=eb_sb, in_=eb.ap())
            ident_sb = const_pool.tile([128, 128], f32)
            nc.sync.dma_start(out=ident_sb, in_=ident.ap())

            for s in range(n_slabs):
                x_slab = x_pool.tile([128, PP * 128], f32)
                nc.sync.dma_start(out=x_slab, in_=x_t[s])
                out_slab = o_pool.tile([128, PP * 128], f32)
                for m in range(PP // 4):          # 4 pairs per psO bank
                    psO4 = psO_pool.tile([128, 512], f32)
                    for h in range(2):            # 2 pairs per psT half-bank
                        psT2 = psT_pool.tile([128, 256], f32)
                        for q in range(2):
                            n = m * 4 + h * 2 + q
                            nc.tensor.transpose(
                                psT2[:, q * 128:(q + 1) * 128],
                                x_slab[:, n * 128:(n + 1) * 128],
                                ident_sb)
                        wt2 = w_pool.tile([128, 256], f32)
                        nc.vector.tensor_copy(wt2, psT2)
                        for q in range(2):
                            nloc = h * 2 + q
                            nc.tensor.matmul(
                                psO4[:, nloc * 128:(nloc + 1) * 128],
                                wt2[:, q * 128:(q + 1) * 128],
                                eb_sb,
                                start=True, stop=True)
                    nc.scalar.copy(
                        out_slab[:, m * 512:(m + 1) * 512], psO4)
                nc.scalar.dma_start(out=y_t[s], in_=out_slab)

    return nc


def _build_nc_v3(rows=ROWS, pairs_per_slab=32, x_bufs=3, o_bufs=3):
    """v3: E = I + D decomposition.  out = x + x_f16 @ D_f16.

    Since Dt*K has tiny norm, D = E - I has entries ~1e-3, so the correction
    term x@D only needs ~f16 precision for ~1e-5 relative output error, while
    x itself passes through exactly (f32 add on DVE).  This turns the PE work
    into a single fp16 matmul per 256 rows (1 cyc/row + FWL weight loads)
    instead of the fp32 multi-pass path, and the transposes move to the DMA
    xbar (2-byte dtype), eliminating all psum->sbuf transpose copies.

    Per 4-pair group (1024 rows):
      - DVE cast:   x16 = f16(x_slab slice)          [128, 512]
      - xbar DMA:   wt16[p, q, f] = x16[f, q*128+p]  (4 block transposes)
      - PE:         psO4[:, q*128:+128] = wt16[:,q,:].T @ blockdiag(D, D)
      - DVE add:    out_slab slice = x_slab slice + psO4   (psum read fused)
    """
    import concourse.mybir as mybir
    from concourse import bacc
    from concourse.tile import TileContext

    f32 = mybir.dt.float32
    f16 = mybir.dt.float16
    nc = bacc.Bacc("TRN2")

    x = nc.dram_tensor("x", [rows, UNITS], f32, kind="ExternalInput")
    db = nc.dram_tensor("db", [128, 128], f16, kind="ExternalInput")
    y = nc.dram_tensor("y", [rows, UNITS], f32, kind="ExternalOutput")

    PP = pairs_per_slab
    n_slabs = rows // (PP * 256)
    assert n_slabs * PP * 256 == rows
    assert PP % 4 == 0

    # row = ((s*PP + n)*128 + p)*2 + r
    x_t = x.ap().rearrange("(s n p r) u -> s p n (r u)", s=n_slabs, n=PP, p=128, r=2)
    y_t = y.ap().rearrange("(s n p r) u -> s p n (r u)", s=n_slabs, n=PP, p=128, r=2)

    with TileContext(nc) as tc:
        with (
            tc.tile_pool(name="const", bufs=1) as const_pool,
            tc.tile_pool(name="xs", bufs=x_bufs) as x_pool,
            tc.tile_pool(name="os", bufs=o_bufs) as o_pool,
            tc.tile_pool(name="x16", bufs=4) as x16_pool,
            tc.tile_pool(name="wt", bufs=4) as w_pool,
            tc.tile_pool(name="pso", bufs=4, space="PSUM") as psO_pool,
        ):
            db_sb = const_pool.tile([128, 128], f16)
            nc.sync.dma_start(out=db_sb, in_=db.ap())

            for s in range(n_slabs):
                x_slab = x_pool.tile([128, PP * 128], f32)
                nc.sync.dma_start(out=x_slab, in_=x_t[s])
                out_slab = o_pool.tile([128, PP * 128], f32)
                for m in range(PP // 4):          # 4 pairs per group
                    xsl = x_slab[:, m * 512:(m + 1) * 512]
                    x16 = x16_pool.tile([128, 512], f16)
                    nc.vector.tensor_copy(x16, xsl)
                    wt16 = w_pool.tile([128, 512], f16)
                    nc.sync.dma_start_transpose(
                        wt16.rearrange("p (q f) -> p q f", q=4), x16)
                    psO4 = psO_pool.tile([128, 512], f32)
                    for q in range(4):
                        nc.tensor.matmul(
                            psO4[:, q * 128:(q + 1) * 128],
                            wt16[:, q * 128:(q + 1) * 128],
                            db_sb,
                            start=True, stop=True)
                    nc.vector.tensor_add(
                        out_slab[:, m * 512:(m + 1) * 512], xsl, psO4)
                nc.scalar.dma_start(out=y_t[s], in_=out_slab)

    return nc


def _build_nc_v3b(rows=ROWS, pairs_per_slab=16, x_bufs=6, o_bufs=4):
    """v3b: like v3 (out = x + x_f16 @ D_f16) but transposes on the PE in
    f16 (1 cyc/row) instead of the DMA xbar (which serialized the sync
    queue and added 16 MiB of fabric traffic).  Engine assignment spreads
    the elementwise work: GpSimd casts, ScalarE copies the transposed f16
    out of psum, VectorE does the fused x + psum add.
    """
    import concourse.mybir as mybir
    from concourse import bacc
    from concourse.tile import TileContext

    f32 = mybir.dt.float32
    f16 = mybir.dt.float16
    nc = bacc.Bacc("TRN2")

    x = nc.dram_tensor("x", [rows, UNITS], f32, kind="ExternalInput")
    db = nc.dram_tensor("db", [128, 128], f16, kind="ExternalInput")
    ident = nc.dram_tensor("ident", [128, 128], f16, kind="ExternalInput")
    y = nc.dram_tensor("y", [rows, UNITS], f32, kind="ExternalOutput")

    PP = pairs_per_slab
    n_slabs = rows // (PP * 256)
    assert n_slabs * PP * 256 == rows
    assert PP % 4 == 0
    G = PP // 4        # [128, 512]-groups per slab; 1024 rows per group

    # row = ((s*G + g)*128 + p)*8 + r  (8 rows per partition):
    # 2 KiB contiguous DRAM per partition per group (line-rate descriptors).
    x_t = x.ap().rearrange("(s g p r) u -> s p g (r u)", s=n_slabs, g=G, p=128, r=8)
    y_t = y.ap().rearrange("(s g p r) u -> s p g (r u)", s=n_slabs, g=G, p=128, r=8)

    with TileContext(nc) as tc:
        with (
            tc.tile_pool(name="const", bufs=1) as const_pool,
            tc.tile_pool(name="xs", bufs=x_bufs) as x_pool,
            tc.tile_pool(name="os", bufs=o_bufs) as o_pool,
            tc.tile_pool(name="x16", bufs=4) as x16_pool,
            tc.tile_pool(name="wt", bufs=4) as w_pool,
            tc.tile_pool(name="pst", bufs=3, space="PSUM") as psT_pool,
            tc.tile_pool(name="pso", bufs=3, space="PSUM") as psO_pool,
        ):
            db_sb = const_pool.tile([128, 128], f16)
            nc.sync.dma_start(out=db_sb, in_=db.ap())
            ident_sb = const_pool.tile([128, 128], f16)
            nc.sync.dma_start(out=ident_sb, in_=ident.ap())

            for s in range(n_slabs):
                x_slab = x_pool.tile([128, PP * 128], f32)
                nc.sync.dma_start(out=x_slab, in_=x_t[s])
                out_slab = o_pool.tile([128, PP * 128], f32)
                for m in range(G):                # 1024 rows per group
                    xsl = x_slab[:, m * 512:(m + 1) * 512]
                    x16 = x16_pool.tile([128, 512], f16)
                    # GpSimd casts are ~3.5x slower than DVE; give it 1 in 4
                    cast_eng = nc.gpsimd if m % 4 == 3 else nc.vector
                    cast_eng.tensor_copy(x16, xsl)
                    psT16 = psT_pool.tile([128, 512], f16)
                    for q in range(4):
                        nc.tensor.transpose(
                            psT16[:, q * 128:(q + 1) * 128],
                            x16[:, q * 128:(q + 1) * 128],
                            ident_sb)
                    wt16 = w_pool.tile([128, 512], f16)
                    nc.scalar.copy(wt16, psT16)
                    psO4 = psO_pool.tile([128, 512], f32)
                    for q in range(4):
                        nc.tensor.matmul(
                            psO4[:, q * 128:(q + 1) * 128],
                            wt16[:, q * 128:(q + 1) * 128],
                            db_sb,
                            start=True, stop=True)
                    nc.vector.tensor_add(
                        out_slab[:, m * 512:(m + 1) * 512], xsl, psO4)
                nc.scalar.dma_start(out=y_t[s], in_=out_slab)

    return nc


VARIANT = "3b"


_BUILDERS = {2: _build_nc_v2, 3: _build_nc_v3}


def _get_nc():
    if "nc" not in _CACHE:
        nc = _BUILDERS.get(VARIANT, _build_nc_v3b)()
        nc.finalize()
        _CACHE["nc"] = nc
    return _CACHE["nc"]


def kernel(inputs, diag, off_diags, Dt, _trace=False):
    from concourse.bass_utils import run_bass_kernel_spmd

    E = _build_E(diag, off_diags, Dt)                       # (64, 64) f32
    x = np.ascontiguousarray(inputs.astype(np.float32, copy=False))
    nc = _get_nc()

    if VARIANT in (3, "3b"):
        D = E.astype(np.float64) - np.eye(64)
        db_np = np.zeros((128, 128), dtype=np.float16)      # blockdiag(D, D)
        db_np[:64, :64] = D.astype(np.float16)
        db_np[64:, 64:] = D.astype(np.float16)
        in_maps = [
            {"x": x[i * ROWS:(i + 1) * ROWS], "db": db_np}
            for i in range(N_CORES)
        ]
        if VARIANT == "3b":
            ident_np = np.eye(128, dtype=np.float16)
            for m in in_maps:
                m["ident"] = ident_np
    else:
        eb_np = np.zeros((128, 128), dtype=np.float32)      # blockdiag(E, E)
        eb_np[:64, :64] = E
        eb_np[64:, 64:] = E
        ident_np = np.eye(128, dtype=np.float32)
        in_maps = [
            {"x": x[i * ROWS:(i + 1) * ROWS], "eb": eb_np, "ident": ident_np}
            for i in range(N_CORES)
        ]

    res = run_bass_kernel_spmd(nc, in_maps, core_ids=list(range(N_CORES)),
                               trace=_trace)
    out = np.concatenate([r["y"] for r in res.results], axis=0)
    if _trace:
        _CACHE["last_results"] = res
    return out
